# revision 5
# baseline (speedup 1.0000x reference)
"""GAT encoder (3-layer, 4-head) on 8 Trainium2 NeuronCores.

Strategy:
- Algebraic collapse: eft=ef@We is only used via ee=sum(eft*ae), so
  ee = efeat @ (eenc_w @ We[l] @ blockcols(ae[l]))  -- computed on host (tiny).
  Likewise el/er fold into 8 extra columns of the node matmul.
- Nodes sharded 6250/core (padded 6272=49*128). Edges sharded by dst owner and
  sorted by dst, tiled to the 128-node tile of their dst, padded to K_CH chunks
  of 128 edges per tile => segment softmax/sums become per-tile PE matmuls with
  an is_equal one-hot built on DVE.
- Softmax max-subtraction dropped (logits are O(0.3); exp cannot overflow, and
  softmax is shift-invariant). Normalization applied after aggregation:
  agg = (onehot.T @ [a*feat | a]); x = agg[:, :256] / agg[:, 256:260].
- feat/el for edge endpoints fetched via per-edge indirect DMA from a
  bf16 [50176, 260] table assembled with one AllGather per layer.
  er[dst] via a second tiny indirect gather from a core-local [6272,4] table.
"""

import numpy as np
import ml_dtypes

N, E = 50000, 800000
IN, EIN, HID, OUT = 128, 64, 256, 256
L, H = 3, 4
D = HID // H
C = 8
NPC = N // C            # 6250
T = 49                  # node tiles per core
NPAD = T * 128          # 6272
P = 128
NTBL = C * NPAD         # 50176 table rows
W = HID + H             # 260 table row width
EPS_SS = 1e-20

_cache = {}


def _host_prep(inputs):
    src = np.asarray(inputs["src"]).astype(np.int64)
    dst = np.asarray(inputs["dst"]).astype(np.int64)
    efeat = np.asarray(inputs["efeat"], dtype=np.float32)
    We = np.asarray(inputs["We"], dtype=np.float32)
    ae = np.asarray(inputs["ae"], dtype=np.float32)
    eenc_w = np.asarray(inputs["eenc_w"], dtype=np.float32)
    eenc_b = np.asarray(inputs["eenc_b"], dtype=np.float32)

    # ee[l, e, h] = (efeat @ eenc_w + eenc_b) @ We[l] . ae[l]  (collapsed)
    V = np.stack([
        np.stack([We[l][:, h0 * D:(h0 + 1) * D] @ ae[l, h0] for h0 in range(H)], 1)
        for l in range(L)])                       # [L, HID, H]
    U = np.stack([eenc_w @ V[l] for l in range(L)])            # [L, EIN, H]
    cvec = np.stack([eenc_b @ V[l] for l in range(L)])         # [L, H]
    ee_all = np.stack([efeat @ U[l] + cvec[l] for l in range(L)])  # [L, E, H] f32

    owner = dst // NPC
    dloc = dst - owner * NPC                      # 0..6249
    tile = dloc >> 7                              # 0..48
    key = (owner * T + tile)
    order = np.argsort(owner * NPAD + dloc, kind="stable")
    cnt = np.bincount(key, minlength=C * T)
    K_CH = int(np.ceil(cnt.max() / P))
    ECH = T * K_CH
    E_PAD = ECH * P

    # rank of each edge within its (core,tile) group (edges are 'order'-sorted)
    ko = key[order]
    starts = np.zeros(C * T, np.int64)
    starts[1:] = np.cumsum(cnt)[:-1]
    rank = np.arange(E, dtype=np.int64) - starts[ko]
    slot = ko % T * (K_CH * P) + rank             # slot within core
    core_of = ko // T
    pp = slot % P
    kk = slot // P                                # chunk index within core

    sg = np.zeros((C, P, ECH), np.int32)
    dg = np.zeros((C, P, ECH), np.int32)
    dr = np.full((C, P, ECH), 255.0, np.float32)
    eeh = np.zeros((C, L, P, ECH * H), np.float32)

    src_o = src[order]
    src_g = (src_o // NPC) * NPAD + (src_o % NPC)
    dloc_o = dloc[order]
    tile_o = tile[order]
    sg[core_of, pp, kk] = src_g.astype(np.int32)
    dg[core_of, pp, kk] = dloc_o.astype(np.int32)
    # padding dg must be a valid row: default 0 is fine
    dr[core_of, pp, kk] = (dloc_o - (tile_o << 7)).astype(np.float32)
    for l in range(L):
        eel = ee_all[l][order]                    # [E, H]
        for h0 in range(H):
            eeh[core_of, l, pp, kk * H + h0] = eel[:, h0]

    # node-side
    h_in = np.asarray(inputs["h"], dtype=np.float32)
    hT = np.zeros((C, IN, NPAD), np.float32)
    for c in range(C):
        hT[c, :, :NPC] = h_in[c * NPC:(c + 1) * NPC].T

    Wn = np.asarray(inputs["Wn"], dtype=np.float32)
    al = np.asarray(inputs["al"], dtype=np.float32)
    ar = np.asarray(inputs["ar"], dtype=np.float32)
    W_cat = np.stack([np.concatenate([
        Wn[l],
        np.stack([Wn[l][:, h0 * D:(h0 + 1) * D] @ al[l, h0] for h0 in range(H)], 1),
        np.stack([Wn[l][:, h0 * D:(h0 + 1) * D] @ ar[l, h0] for h0 in range(H)], 1),
    ], axis=1) for l in range(L)])                # [L, 256, 264]

    prep = {
        "K_CH": K_CH, "ECH": ECH, "E_PAD": E_PAD,
        "sg": sg, "dg": dg, "dr": dr, "eeh": eeh, "hT": hT,
        "W_cat": W_cat.astype(ml_dtypes.bfloat16),
        "enc_w": np.asarray(inputs["enc_w"], np.float32),
        "enc_b_rep": np.tile(np.asarray(inputs["enc_b"], np.float32)[None, :], (P, 1)),
        "out_w": np.asarray(inputs["out_w"], np.float32).astype(ml_dtypes.bfloat16),
        "out_b_rep": np.tile(np.asarray(inputs["out_b"], np.float32)[None, :], (P, 1)),
        "ln_g_rep": np.tile(np.asarray(inputs["ln_g"], np.float32)[:, None, :], (1, P, 1)),
        "ln_b_rep": np.tile(np.asarray(inputs["ln_b"], np.float32)[:, None, :], (1, P, 1)),
        "iota": np.tile(np.arange(P, dtype=np.float32)[None, :], (P, 1)),
        "alpha02": np.full((P, 1), 0.2, np.float32),
    }
    return prep


def _build_nc(K_CH):
    import concourse.bass as bass
    import concourse.mybir as mybir
    import concourse.tile as tile

    F32 = mybir.dt.float32
    BF16 = mybir.dt.bfloat16
    I32 = mybir.dt.int32
    ECH = T * K_CH

    nc = bass.Bass()
    dp = nc.declare_dram_parameter
    hT_in = dp("hT", [IN, NPAD], F32, isOutput=False)
    sg_in = dp("sg", [P, ECH], I32, isOutput=False)
    dg_in = dp("dg", [P, ECH], I32, isOutput=False)
    dr_in = dp("dr", [P, ECH], F32, isOutput=False)
    ee_in = dp("ee", [L, P, ECH * H], F32, isOutput=False)
    encw_in = dp("enc_w", [IN, HID], F32, isOutput=False)
    encb_in = dp("enc_b_rep", [P, HID], F32, isOutput=False)
    wcat_in = dp("W_cat", [L, HID, HID + 2 * H], BF16, isOutput=False)
    outw_in = dp("out_w", [HID, OUT], BF16, isOutput=False)
    outb_in = dp("out_b_rep", [P, OUT], F32, isOutput=False)
    lng_in = dp("ln_g_rep", [L, P, HID], F32, isOutput=False)
    lnb_in = dp("ln_b_rep", [L, P, HID], F32, isOutput=False)
    iota_in = dp("iota", [P, P], F32, isOutput=False)
    al_in = dp("alpha02", [P, 1], F32, isOutput=False)
    y_out = dp("y", [NPAD, OUT], F32, isOutput=True)

    AFT = mybir.ActivationFunctionType
    ALU = mybir.AluOpType

    def ap(tl, dims):
        b = tl[:]
        return bass.AP(b.tensor, b.offset, dims)

    def apo(tl, off, dims):
        b = tl[:]
        return bass.AP(b.tensor, b.offset + off, dims)

    with tile.TileContext(nc) as tc:
        with tc.tile_pool(name="dram", bufs=1, space="DRAM") as dram, \
             tc.tile_pool(name="const", bufs=1) as cst, \
             tc.tile_pool(name="sbuf", bufs=3) as sb, \
             tc.tile_pool(name="sgbuf", bufs=2) as gb, \
             tc.tile_pool(name="psum", bufs=2, space="PSUM") as pp:

            table_in = dram.tile([NPAD, W], BF16)
            table = dram.tile([NTBL, W], BF16)
            er_dram = dram.tile([NPAD, H], BF16)
            x_dram = [dram.tile([NPAD, HID], F32, name=f"x{i}", tag=f"x{i}")
                      for i in range(2)]
            xbf_dram = dram.tile([NPAD, HID], BF16)

            # resident constants
            sg = cst.tile([P, ECH], I32)
            dg = cst.tile([P, ECH], I32)
            dr = cst.tile([P, ECH], F32)
            iota = cst.tile([P, P], F32)
            al02 = cst.tile([P, 1], F32)
            hTs = cst.tile([IN, NPAD], F32)
            encw = cst.tile([IN, HID], F32)
            encb = cst.tile([P, HID], F32)
            wcat = cst.tile([P, L, 2, HID + 2 * H], BF16)
            outw = cst.tile([P, 2, OUT], BF16)
            outb = cst.tile([P, OUT], F32)
            lng = cst.tile([P, L, HID], F32)
            lnb = cst.tile([P, L, HID], F32)
            for t_, s_ in [(sg, sg_in), (dg, dg_in), (dr, dr_in), (iota, iota_in),
                           (al02, al_in), (hTs, hT_in), (encw, encw_in),
                           (encb, encb_in), (outb, outb_in)]:
                nc.sync.dma_start(out=t_[:], in_=s_[:])
            nc.sync.dma_start(
                out=wcat[:],
                in_=wcat_in[:].rearrange("l (k p) w -> p l k w", p=P))
            nc.sync.dma_start(
                out=outw[:], in_=outw_in[:].rearrange("(k p) w -> p k w", p=P))
            nc.sync.dma_start(out=lng[:], in_=lng_in[:].rearrange("l p w -> p l w"))
            nc.sync.dma_start(out=lnb[:], in_=lnb_in[:].rearrange("l p w -> p l w"))

            # ---------------- encoder: x0 = h @ enc_w + enc_b ----------------
            for t in range(T):
                ps = pp.tile([P, HID], F32, tag="ps_enc")
                nc.tensor.matmul(out=ps[:], lhsT=hTs[:, t * P:(t + 1) * P],
                                 rhs=encw[:], start=True, stop=True)
                xt = sb.tile([P, HID], F32, tag="xt_enc")
                nc.vector.tensor_tensor(out=xt[:], in0=ps[:], in1=encb[:], op=ALU.add)
                nc.sync.dma_start(out=x_dram[0][t * P:(t + 1) * P, :], in_=xt[:])
                xb = sb.tile([P, HID], BF16, tag="xb_enc")
                nc.any.tensor_copy(out=xb[:], in_=xt[:])
                nc.sync.dma_start(out=xbf_dram[t * P:(t + 1) * P, :], in_=xb[:])

            for l in range(L):
                x_cur = x_dram[l % 2]
                x_nxt = x_dram[(l + 1) % 2]
                # -------- node matmul: [feat | el | er] = x @ W_cat[l] --------
                xT = cst.tile([P, 2, NPAD], BF16, tag="xT")
                for k in range(2):
                    nc.sync.dma_start_transpose(
                        out=xT[:, k, :], in_=xbf_dram[:, k * P:(k + 1) * P])
                for t in range(T):
                    ps = pp.tile([P, HID + 2 * H], F32, tag="ps_node")
                    for k in range(2):
                        nc.tensor.matmul(out=ps[:], lhsT=xT[:, k, t * P:(t + 1) * P],
                                         rhs=wcat[:, l, k, :],
                                         start=(k == 0), stop=(k == 1))
                    fe = sb.tile([P, HID + 2 * H], BF16, tag="fe")
                    nc.any.tensor_copy(out=fe[:], in_=ps[:])
                    nc.sync.dma_start(out=table_in[t * P:(t + 1) * P, :],
                                      in_=fe[:, 0:W])
                    nc.sync.dma_start(out=er_dram[t * P:(t + 1) * P, :],
                                      in_=fe[:, W:W + H])
                # -------- allgather the [feat|el] table --------
                nc.gpsimd.collective_compute(
                    "AllGather", mybir.AluOpType.bypass,
                    replica_groups=[list(range(C))],
                    ins=[table_in.opt()], outs=[table.opt()])
                ee_sb = cst.tile([P, ECH * H], F32, tag="ee_sb")
                nc.sync.dma_start(out=ee_sb[:], in_=ee_in[l, :, :])

                # -------- edge stage --------
                for t in range(T):
                    c0 = t * K_CH
                    G = gb.tile([P, K_CH, W], BF16, tag="G")
                    erg = gb.tile([P, K_CH, H], BF16, tag="erg")
                    for k in range(K_CH):
                        nc.gpsimd.indirect_dma_start(
                            out=G[:, k, :], out_offset=None, in_=table[:],
                            in_offset=bass.IndirectOffsetOnAxis(
                                ap=sg[:, c0 + k:c0 + k + 1], axis=0))
                        nc.gpsimd.indirect_dma_start(
                            out=erg[:, k, :], out_offset=None, in_=er_dram[:],
                            in_offset=bass.IndirectOffsetOnAxis(
                                ap=dg[:, c0 + k:c0 + k + 1], axis=0))
                    oh = gb.tile([P, K_CH, P], BF16, tag="oh")
                    p_dr, p_io = dr[:].ap[0][0], iota[:].ap[0][0]
                    nc.vector.tensor_tensor(
                        out=oh[:],
                        in0=apo(dr, c0, [[p_dr, P], [1, K_CH], [0, P]]),
                        in1=ap(iota, [[p_io, P], [0, K_CH], [1, P]]),
                        op=ALU.is_equal)
                    # logits -> a
                    p_g, p_e = G[:].ap[0][0], erg[:].ap[0][0]
                    e1 = sb.tile([P, K_CH * H], F32, tag="e1")
                    nc.vector.tensor_tensor(
                        out=e1[:], in0=ap(erg, [[p_e, P], [1, K_CH * H]]),
                        in1=ee_sb[:, c0 * H:(c0 + K_CH) * H], op=ALU.add)
                    el_ap = apo(G, HID, [[p_g, P], [W, K_CH], [1, H]])
                    nc.vector.tensor_tensor(out=e1[:], in0=e1[:], in1=el_ap, op=ALU.add)
                    nc.scalar.activation(out=e1[:], in_=e1[:], func=AFT.Prelu,
                                         alpha=al02[:])
                    a_bf = sb.tile([P, K_CH * H], BF16, tag="a_bf")
                    nc.scalar.activation(out=a_bf[:], in_=e1[:], func=AFT.Exp)
                    # vals = [a (x) feat | a]
                    vals = gb.tile([P, K_CH, W], BF16, tag="vals")
                    p_v, p_a = vals[:].ap[0][0], a_bf[:].ap[0][0]
                    nc.vector.tensor_tensor(
                        out=ap(vals, [[p_v, P], [W, K_CH], [64, H], [1, 64]]),
                        in0=ap(G, [[p_g, P], [W, K_CH], [64, H], [1, 64]]),
                        in1=ap(a_bf, [[p_a, P], [H, K_CH], [1, H], [0, 64]]),
                        op=ALU.mult)
                    nc.any.tensor_copy(
                        out=apo(vals, HID, [[p_v, P], [W, K_CH], [1, H]]),
                        in_=a_bf[:])
                    ps = pp.tile([P, W], F32, tag="ps_agg")
                    for k in range(K_CH):
                        nc.tensor.matmul(out=ps[:], lhsT=oh[:, k, :], rhs=vals[:, k, :],
                                         start=(k == 0), stop=(k == K_CH - 1))
                    # normalize, relu, residual, layernorm
                    rec = sb.tile([P, H], F32, tag="rec")
                    nc.vector.tensor_scalar_max(out=rec[:], in0=ps[:, HID:W],
                                                scalar1=EPS_SS)
                    nc.vector.reciprocal(out=rec[:], in_=rec[:])
                    y2 = sb.tile([P, HID], F32, tag="y2")
                    p_r, p_y = rec[:].ap[0][0], y2[:].ap[0][0]
                    nc.vector.tensor_tensor(
                        out=ap(y2, [[p_y, P], [64, H], [1, 64]]),
                        in0=ap(ps, [[ps[:].ap[0][0], P], [64, H], [1, 64]]),
                        in1=ap(rec, [[p_r, P], [1, H], [0, 64]]),
                        op=ALU.mult)
                    nc.scalar.activation(out=y2[:], in_=y2[:], func=AFT.Relu)
                    res = sb.tile([P, HID], F32, tag="res")
                    nc.sync.dma_start(out=res[:], in_=x_cur[t * P:(t + 1) * P, :])
                    nc.vector.tensor_tensor(out=y2[:], in0=y2[:], in1=res[:], op=ALU.add)
                    s1 = sb.tile([P, 1], F32, tag="s1")
                    sq = sb.tile([P, HID], F32, tag="sq")
                    s2 = sb.tile([P, 1], F32, tag="s2")
                    nc.vector.reduce_sum(out=s1[:], in_=y2[:], axis=mybir.AxisListType.X)
                    nc.vector.tensor_tensor(out=sq[:], in0=y2[:], in1=y2[:], op=ALU.mult)
                    nc.vector.reduce_sum(out=s2[:], in_=sq[:], axis=mybir.AxisListType.X)
                    mu = sb.tile([P, 1], F32, tag="mu")
                    nc.vector.tensor_scalar_mul(out=mu[:], in0=s1[:], scalar1=1.0 / HID)
                    var = sb.tile([P, 1], F32, tag="var")
                    nc.vector.tensor_scalar_mul(out=var[:], in0=s2[:], scalar1=1.0 / HID)
                    musq = sb.tile([P, 1], F32, tag="musq")
                    nc.vector.tensor_tensor(out=musq[:], in0=mu[:], in1=mu[:], op=ALU.mult)
                    nc.vector.tensor_tensor(out=var[:], in0=var[:], in1=musq[:],
                                            op=ALU.subtract)
                    nc.vector.tensor_scalar_add(out=var[:], in0=var[:], scalar1=1e-5)
                    rstd = sb.tile([P, 1], F32, tag="rstd")
                    nc.scalar.activation(out=rstd[:], in_=var[:], func=AFT.Sqrt)
                    nc.vector.reciprocal(out=rstd[:], in_=rstd[:])
                    xn = sb.tile([P, HID], F32, tag="xn")
                    nc.vector.tensor_scalar(out=xn[:], in0=y2[:], scalar1=mu[:],
                                            scalar2=rstd[:], op0=ALU.subtract,
                                            op1=ALU.mult)
                    nc.vector.tensor_tensor(out=xn[:], in0=xn[:], in1=lng[:, l, :],
                                            op=ALU.mult)
                    nc.vector.tensor_tensor(out=xn[:], in0=xn[:], in1=lnb[:, l, :],
                                            op=ALU.add)
                    if l < L - 1:
                        nc.sync.dma_start(out=x_nxt[t * P:(t + 1) * P, :], in_=xn[:])
                    xb = sb.tile([P, HID], BF16, tag="xb_l")
                    nc.any.tensor_copy(out=xb[:], in_=xn[:])
                    nc.sync.dma_start(out=xbf_dram[t * P:(t + 1) * P, :], in_=xb[:])

            # ---------------- output: y = x3 @ out_w + out_b ----------------
            xT = cst.tile([P, 2, NPAD], BF16, tag="xT")
            for k in range(2):
                nc.sync.dma_start_transpose(
                    out=xT[:, k, :], in_=xbf_dram[:, k * P:(k + 1) * P])
            for t in range(T):
                ps = pp.tile([P, OUT], F32, tag="ps_out")
                for k in range(2):
                    nc.tensor.matmul(out=ps[:], lhsT=xT[:, k, t * P:(t + 1) * P],
                                     rhs=outw[:, k, :], start=(k == 0), stop=(k == 1))
                yt = sb.tile([P, OUT], F32, tag="yt")
                nc.vector.tensor_tensor(out=yt[:], in0=ps[:], in1=outb[:], op=ALU.add)
                nc.sync.dma_start(out=y_out[t * P:(t + 1) * P, :], in_=yt[:])

    _split_sync_waits(nc)
    return nc


def _split_sync_waits(nc, max_waits=1):
    """walrus on this toolchain accepts at most one semaphore wait per
    instruction; hoist extras onto preceding same-engine NoOps."""
    import concourse.mybir as mybir
    ctr = 0
    for f in nc.m.functions:
        for bb in f.blocks:
            new_list = []
            for ins in bb.instructions:
                si = ins.sync_info
                if si is not None and si.on_wait and len(si.on_wait) > max_waits:
                    waits = list(si.on_wait)
                    head, tail = waits[:-max_waits], waits[-max_waits:]
                    while head:
                        chunk, head = head[:max_waits], head[max_waits:]
                        ctr += 1
                        nop = mybir.InstNoOp(name=f"wsplit{ctr}_{ins.name}",
                                             engine=ins.engine)
                        nop.sync_info = mybir.SyncInfo(on_wait=chunk, on_update=[])
                        new_list.append(nop)
                    ins.sync_info = mybir.SyncInfo(
                        on_wait=tail, on_update=list(si.on_update or []))
                new_list.append(ins)
            try:
                bb.instructions = new_list
            except Exception:
                bb.instructions.clear()
                for i in new_list:
                    bb.instructions.append(i)


def _get_compiled(prep):
    key = prep["K_CH"]
    if key not in _cache:
        _cache[key] = _build_nc(key)
    return _cache[key]


def make_in_maps(prep):
    bf16 = ml_dtypes.bfloat16
    shared = {
        "enc_w": prep["enc_w"], "enc_b_rep": prep["enc_b_rep"],
        "W_cat": np.ascontiguousarray(prep["W_cat"]),
        "out_w": np.ascontiguousarray(prep["out_w"]),
        "out_b_rep": prep["out_b_rep"],
        "ln_g_rep": prep["ln_g_rep"], "ln_b_rep": prep["ln_b_rep"],
        "iota": prep["iota"], "alpha02": prep["alpha02"],
    }
    in_maps = []
    for c in range(C):
        m = dict(shared)
        m["hT"] = prep["hT"][c]
        m["sg"] = prep["sg"][c]
        m["dg"] = prep["dg"][c]
        m["dr"] = prep["dr"][c]
        m["ee"] = prep["eeh"][c]
        in_maps.append(m)
    return in_maps


def kernel(**inputs):
    import sys
    for p in ("/opt/trn_rl_repo",):
        if p not in sys.path:
            sys.path.insert(0, p)
    from concourse.bass_utils import run_bass_kernel_spmd

    prep = _host_prep(inputs)
    nc = _get_compiled(prep)
    in_maps = make_in_maps(prep)
    res = run_bass_kernel_spmd(nc, in_maps, core_ids=list(range(C)))
    out = np.concatenate(
        [res.results[c]["y"][:NPC] for c in range(C)], axis=0)
    return out.astype(np.float32)
